# revision 33
# baseline (speedup 1.0000x reference)
"""Bidirectional Mamba block on 8 Trainium2 NeuronCores.

Strategy (v2)
-------------
Data-parallel over batch: each core runs one batch element (both
directions); no collectives.

Per core / per direction, d_inner=512 as 4 blocks of 128 partitions,
n=16 states, L=4096 in 2 halves of HL=2048:

  phase 1 (PE+ACT): x_conv = silu(conv1d(u @ in_w_x.T) + conv_b) with the
          depthwise conv folded into the input projection (4 taps x 2
          d_model tiles of host-combined weights); sz = silu(u @ in_w_z.T)
          bounced to DRAM until gating.
  phase 2 (PE+ACT): dbl = x_conv @ xproj.T with host-permuted rows so each
          state group's B/C rows are adjacent; delta = softplus(dt-proj +
          dt_b) via Exp+Ln (one ACT table set). B/C rows land in a small
          SBUF tile (bcsb) and are broadcast to 128 partitions by
          SBUF->SBUF DMA per (group, j) pair.
  phase 3: per (block, n): a = exp(A[:,n] * delta) on ACT at FD=2048;
          b = (delta*x_conv) * B (DVE, partly GPSIMD); h =
          tensor_tensor_scan(a, b) at FD=2048 chained across halves via
          f32 state tiles; p = h * C on GPSIMD; the n-reduction
          y = sum_n p runs on the PE as identity matmuls accumulating in
          PSUM (start/stop over the 16 planes).
  gating: gt = ((x_conv * D) + y_psum) * sz via one scalar_tensor_tensor
          + one tensor_tensor; out-proj on the PE (0.5 folded into out_w).

The backward direction runs first on a host-reversed input copy; its
out-proj result is stored to DRAM (f32) and combined, reversed, with the
forward direction's out-proj.
"""

import contextlib
import functools
import sys

for _p in ("/opt/trn_rl_repo",):
    if _p not in sys.path:
        sys.path.insert(0, _p)

import numpy as np

import concourse.bass as bass
import concourse.bacc as bacc
import concourse.mybir as mybir
import concourse.tile as tile

F16 = mybir.dt.float16
F32 = mybir.dt.float32
AOP = mybir.AluOpType
ACT = mybir.ActivationFunctionType

D_MODEL = 256
D_INNER = 512
NSTATE = 16
DT_RANK = 16
KCONV = 4
NBLK = D_INNER // 128   # 4 d_inner blocks
NMT = D_MODEL // 128    # 2 d_model tiles
NJ = 4                  # n per group
NG = NSTATE // NJ       # 4 groups
N_CORES = 8

L_FULL = 4096
T = 512                 # PSUM-sized chunk
N_CORES = 8

# which (g, j) bt muls go to GPSIMD instead of DVE (pt2 is always GPSIMD)
BT_GPSIMD_J = ()

SILU_NATIVE = True


def _patch_act_tables():
    """Keep Exp and Ln resolving to natural_log_exp_and_others so softplus
    (Exp+Ln) and the a-gen Exps never force ACT table reloads."""
    import concourse.bacc as _bacc
    import concourse.hw_specs as _hw

    if getattr(_bacc, "_mamba_act_patch", False):
        return
    real = _hw.get_activation_tables

    def patched(arch):
        tabs = dict(real(arch))
        keep = ("natural_log_exp_and_others", "silu_and_others")
        for nm in list(tabs):
            if nm not in keep:
                tabs[nm] = set()
        return tabs

    _bacc.get_activation_tables = patched
    _bacc._mamba_act_patch = True


def _emit_dir(nc, tc, pools, prm, W, dirn, lh, L, ob, y_param):
    HL = L // 2
    NCH = HL // T
    h0 = lh * HL
    wp, hp, core, ps, psy, sm, dram = (
        pools["weights"], pools["half"], pools["core"], pools["psum"],
        pools["psumy"], pools["small"], pools["dram"],
    )

    def dma(out, in_):
        nc.sync.dma_start(out=out, in_=in_)

    wx, wz, wxp, wdt, wo, cb, dtb, At, Dd, ident, state = (
        W['wx'], W['wz'], W['wxp'], W['wdt'], W['wo'], W['cb'], W['dtb'],
        W['A'], W['D'], W['ident'], W['state'])
    Ddg = W['Ddg']

    u_param = prm[f"u_{dirn}"]

    # ---- per-(direction, half) tensors; bufs=2 pipelines the two halves ----
    xc = [hp.tile([128, HL], F16, tag=f"xc{b}", name=f"xc{b}", bufs=2) for b in range(NBLK)]
    dl = [hp.tile([128, HL], F16, tag=f"dl{b}", name=f"dl{b}", bufs=2) for b in range(NBLK)]
    wh = [hp.tile([128, HL], F16, tag=f"wh{b}", name=f"wh{b}", bufs=2) for b in range(NBLK)]
    szd = [dram.tile([128, HL], F16, tag=f"szd{b}", name=f"szd{b}", bufs=2) for b in range(NBLK)]
    bcsb = hp.tile([2 * NSTATE, HL], F16, tag="bcsb", name="bcsb", bufs=2)
    bc = dram.tile([2 * NSTATE, HL], F16, tag="bc", name="bc", bufs=2)

    # ---- phase 1: x_conv (conv folded into in-proj); z lags one chunk ----
    def _emit_z(item):
        uu, s0 = item
        for b in range(NBLK):
            pz = ps.tile([128, T], F32, tag="pz", name="pz", bufs=1)
            for dmb in range(2):
                nc.tensor.matmul(
                    pz[:], wz[dmb][:, b * 128:(b + 1) * 128],
                    uu[dmb][:, KCONV - 1:KCONV - 1 + T],
                    start=(dmb == 0), stop=(dmb == 1),
                )
            szc = core.tile([128, T], F16, tag="szc", name="szc", bufs=1)
            nc.scalar.activation(out=szc[:], in_=pz[:], func=ACT.Silu)
            dma(szd[b][:, s0:s0 + T], szc[:])

    for ci in range(NCH):
        t0 = h0 + ci * T
        s0 = ci * T
        u0 = core.tile([128, T + KCONV - 1], F16, tag="u0", name="u0")
        u1 = core.tile([128, T + KCONV - 1], F16, tag="u1", name="u1")
        dma(u0, u_param[0:128, t0:t0 + T + KCONV - 1])
        dma(u1, u_param[128:256, t0:t0 + T + KCONV - 1])
        uu = (u0, u1)
        for b in range(NBLK):
            px = ps.tile([128, T], F32, tag="px", name="px", bufs=1)
            for kb in range(2 * KCONV):
                k, dmb = divmod(kb, 2)
                nc.tensor.matmul(
                    px[:], wx[kb][:, b * 128:(b + 1) * 128],
                    uu[dmb][:, k:k + T],
                    start=(kb == 0), stop=(kb == 2 * KCONV - 1),
                )
            nc.scalar.activation(
                out=xc[b][:, s0:s0 + T], in_=px[:], func=ACT.Silu,
                bias=cb[b][:], scale=1.0,
            )
        _emit_z((uu, s0))

    # ---- phase 2: dbl (dt/B/C rows, B/C host-permuted) and delta ----
    # For the first unit, promote phase 2 so the scheduler interleaves it
    # with phase 1 per-chunk instead of queueing it behind all of phase 1;
    # this shortens the warm-up critical path to the first scan.
    p2ctx = (tc.high_priority() if (dirn == "b" and lh == 0)
             else contextlib.nullcontext())
    with p2ctx:
      for ci in range(NCH):
        s0 = ci * T
        pd = ps.tile([128, T], F32, tag="pd", name="pd")
        for b in range(NBLK):
            nc.tensor.matmul(
                pd[0:DT_RANK + 2 * NSTATE, :], wxp[b][:], xc[b][:, s0:s0 + T],
                start=(b == 0), stop=(b == NBLK - 1),
            )
        dt16 = core.tile([DT_RANK, T], F16, tag="dt16", name="dt16", bufs=1)
        nc.scalar.activation(out=dt16[:],
                             in_=pd[2 * NSTATE:2 * NSTATE + DT_RANK, :],
                             func=ACT.Copy)
        nc.scalar.activation(out=bcsb[:, s0:s0 + T],
                             in_=pd[0:2 * NSTATE, :],
                             func=ACT.Copy)
        for b in range(NBLK):
            pt = ps.tile([128, T], F32, tag="pd", name="pt")
            nc.tensor.matmul(
                pt[:], wdt[:, b * 128:(b + 1) * 128], dt16[:],
                start=True, stop=True,
            )
            et = core.tile([128, T], F32, tag="et", name="et", bufs=1)
            nc.scalar.activation(out=et[:], in_=pt[:], func=ACT.Exp,
                                 bias=dtb[b][:], scale=1.0)
            nc.scalar.activation(out=dl[b][:, s0:s0 + T], in_=et[:],
                                 func=ACT.Ln, bias=1.0, scale=1.0)
        dma(bc[:, s0:s0 + T], bcsb[:, s0:s0 + T])

    # ---- phase 3: selective scan; n-reduce on PE into PSUM ----
    gts = []
    P_SIZES = (HL // 2, HL // 2) if (dirn == "b" and lh == 0) else (HL,)
    NP = len(P_SIZES)
    for b in range(NBLK):
        yacc = [psy.tile([128, T], F32, tag=f"yac{c}", name=f"yac{c}")
                for c in range(NCH)]
        state2 = wp.tile([128, NSTATE], F32, tag=f"st2_{b}", name=f"st2_{b}")
        for p in range(NP):
            PL = P_SIZES[p]
            p0 = sum(P_SIZES[:p])
            for g in range(NG):
                for j in range(NJ):
                    n = g * NJ + j
                    bbt = core.tile([128, HL], F16, tag="bbt", name="bbt",
                                    bufs=2)
                    cbt = core.tile([128, HL], F16, tag="cbt", name="cbt",
                                    bufs=2)
                    at = core.tile([128, HL], F16, tag="at", name="at",
                                   bufs=3)
                    bt = core.tile([128, HL], F16, tag="bt", name="bt",
                                   bufs=2)
                    ht = core.tile([128, HL], F16, tag="ht", name="ht",
                                   bufs=3)
                    pt2 = core.tile([128, HL], F16, tag="pt2", name="pt2",
                                    bufs=1)
                    bc_ap = bc[:]
                    if g == 0 and j == 0:
                        nc.vector.tensor_tensor(
                            wh[b][:, p0:p0 + PL], dl[b][:, p0:p0 + PL],
                            xc[b][:, p0:p0 + PL], AOP.mult)
                    dma(bbt[:, p0:p0 + PL], bass.AP(
                        tensor=bc_ap.tensor,
                        offset=bc_ap.offset + (2 * n) * HL + p0,
                        ap=[[0, 128], [1, PL]],
                    ))
                    dma(cbt[:, p0:p0 + PL], bass.AP(
                        tensor=bc_ap.tensor,
                        offset=bc_ap.offset + (2 * n + 1) * HL + p0,
                        ap=[[0, 128], [1, PL]],
                    ))
                    nc.scalar.activation(
                        out=at[:, p0:p0 + PL], in_=dl[b][:, p0:p0 + PL],
                        func=ACT.Exp, scale=At[b][:, n:n + 1],
                    )
                    nc.vector.tensor_tensor(
                        bt[:, p0:p0 + PL], wh[b][:, p0:p0 + PL],
                        bbt[:, p0:p0 + PL], AOP.mult)
                    if p == 0:
                        init = 0.0 if lh == 0 else state[b][:, n:n + 1]
                    else:
                        init = state2[:, n:n + 1]
                    nc.vector.tensor_tensor_scan(
                        ht[:, p0:p0 + PL], at[:, p0:p0 + PL],
                        bt[:, p0:p0 + PL], init, AOP.mult, AOP.add)
                    if p < NP - 1:
                        nc.gpsimd.tensor_copy(
                            state2[:, n:n + 1],
                            ht[:, p0 + PL - 1:p0 + PL])
                    if lh == 0 and p == NP - 1:
                        nc.gpsimd.tensor_copy(
                            state[b][:, n:n + 1], ht[:, HL - 1:HL])
                    nc.vector.tensor_tensor(
                        pt2[:, p0:p0 + PL], ht[:, p0:p0 + PL],
                        cbt[:, p0:p0 + PL], AOP.mult)
                    for c in range(p0 // T, (p0 + PL) // T):
                        nc.tensor.matmul(
                            yacc[c][:], ident[:], pt2[:, c * T:(c + 1) * T],
                            start=(g == 0 and j == 0), stop=False,
                        )
        # gating for this block: yacc += diag(D) @ xc on PE, then * sz
        gt = core.tile([128, HL], F16, tag=f"gt{b}", name=f"gt{b}", bufs=1)
        szt = core.tile([128, HL], F16, tag="szt", name="szt", bufs=2)
        dma(szt, szd[b][:])
        for c in range(NCH):
            nc.tensor.matmul(
                yacc[c][:], Ddg[b][:], xc[b][:, c * T:(c + 1) * T],
                start=False, stop=True,
            )
            nc.vector.tensor_tensor(
                gt[:, c * T:(c + 1) * T], yacc[c][:],
                szt[:, c * T:(c + 1) * T], AOP.mult)
        gts.append(gt)
    # ---- out-proj ----
    last_unit = (dirn == "f" and lh == 1)
    for c in range(NCH):
        t0 = h0 + c * T
        for mt in range(NMT):
            po_tag = "px" if (last_unit and (c * NMT + mt) % 2 == 1) else "po"
            po = ps.tile([128, T], F32, tag=po_tag, name="po", bufs=1)
            for b in range(NBLK):
                nc.tensor.matmul(
                    po[:], wo[b][:, mt * 128:(mt + 1) * 128],
                    gts[b][:, c * T:(c + 1) * T],
                    start=(b == 0),
                    stop=(b == NBLK - 1 and dirn == "b"),
                )
            if dirn == "b":
                obs = core.tile([128, T], F16, tag="obs", name="obs", bufs=1)
                nc.scalar.activation(out=obs[:], in_=po[:], func=ACT.Copy)
                dma(ob[mt][:, t0:t0 + T], obs[:])
            else:
                cb_rev = (L // T) - 1 - (t0 // T)
                obs = core.tile([128, T], F16, tag="obs", name="obs", bufs=1)
                dma(obs, ob[mt][:, cb_rev * T:(cb_rev + 1) * T])
                nc.tensor.matmul(
                    po[:], ident[:], obs[:, ::-1],
                    start=False, stop=True,
                )
                oo = core.tile([128, T], F32, tag="oo", name="oo", bufs=1)
                nc.scalar.activation(out=oo[:], in_=po[:], func=ACT.Copy)
                dma(y_param[mt * 128:(mt + 1) * 128, t0:t0 + T], oo[:])


def _load_weights(nc, wp, sm, prm, dirn, ident):
    def dma(out, in_):
        nc.sync.dma_start(out=out, in_=in_)
    W = {"ident": ident}
    W["wx"] = []
    for i in range(2 * KCONV):
        t = wp.tile([128, D_INNER], F16, tag=f"wx{i}", name=f"wx{i}")
        dma(t, prm[f"{dirn}_wx"][i])
        W["wx"].append(t)
    W["wz"] = []
    for i in range(2):
        t = wp.tile([128, D_INNER], F16, tag=f"wz{i}", name=f"wz{i}")
        dma(t, prm[f"{dirn}_wz"][i])
        W["wz"].append(t)
    W["wxp"] = []
    for b in range(NBLK):
        t = wp.tile([128, DT_RANK + 2 * NSTATE], F16, tag=f"wxp{b}", name=f"wxp{b}")
        dma(t, prm[f"{dirn}_wxp"][b])
        W["wxp"].append(t)
    W["wdt"] = wp.tile([DT_RANK, D_INNER], F16, tag="wdt", name="wdt")
    dma(W["wdt"], prm[f"{dirn}_wdt"][:])
    W["wo"] = []
    for b in range(NBLK):
        t = wp.tile([128, D_MODEL], F16, tag=f"wo{b}", name=f"wo{b}")
        dma(t, prm[f"{dirn}_wo"][b])
        W["wo"].append(t)
    W["Ddg"] = []
    for b in range(NBLK):
        t = wp.tile([128, 128], F16, tag=f"Ddg{b}", name=f"Ddg{b}")
        dma(t, prm[f"{dirn}_Ddg"][b])
        W["Ddg"].append(t)
    for key, pname, width in (("cb", "cb", 1), ("dtb", "dtb", 1),
                              ("A", "A", NSTATE), ("D", "D", 1)):
        W[key] = []
        for b in range(NBLK):
            t = wp.tile([128, width], F32, tag=f"{key}{b}", name=f"{key}{b}")
            dma(t, prm[f"{dirn}_{pname}"][b])
            W[key].append(t)
    W["state"] = [sm.tile([128, NSTATE], F32, tag=f"st{b}", name=f"st{b}")
                  for b in range(NBLK)]
    return W


def build_nc(L):
    _patch_act_tables()
    nc = bacc.Bacc("TRN2", target_bir_lowering=False, debug=False)
    prm = {}
    prm["u_f"] = nc.declare_dram_parameter("u_f", [D_MODEL, L + KCONV - 1], F16, isOutput=False)
    prm["u_b"] = nc.declare_dram_parameter("u_b", [D_MODEL, L + KCONV - 1], F16, isOutput=False)
    prm["ident"] = nc.declare_dram_parameter("ident", [128, 128], F16, isOutput=False)
    for d in ("f", "b"):
        prm[f"{d}_wx"] = nc.declare_dram_parameter(f"{d}_wx", [2 * KCONV, 128, D_INNER], F16, isOutput=False)
        prm[f"{d}_wz"] = nc.declare_dram_parameter(f"{d}_wz", [2, 128, D_INNER], F16, isOutput=False)
        prm[f"{d}_wxp"] = nc.declare_dram_parameter(f"{d}_wxp", [NBLK, 128, DT_RANK + 2 * NSTATE], F16, isOutput=False)
        prm[f"{d}_wdt"] = nc.declare_dram_parameter(f"{d}_wdt", [DT_RANK, D_INNER], F16, isOutput=False)
        prm[f"{d}_wo"] = nc.declare_dram_parameter(f"{d}_wo", [NBLK, 128, D_MODEL], F16, isOutput=False)
        prm[f"{d}_cb"] = nc.declare_dram_parameter(f"{d}_cb", [NBLK, 128, 1], F32, isOutput=False)
        prm[f"{d}_dtb"] = nc.declare_dram_parameter(f"{d}_dtb", [NBLK, 128, 1], F32, isOutput=False)
        prm[f"{d}_A"] = nc.declare_dram_parameter(f"{d}_A", [NBLK, 128, NSTATE], F32, isOutput=False)
        prm[f"{d}_D"] = nc.declare_dram_parameter(f"{d}_D", [NBLK, 128, 1], F32, isOutput=False)
        prm[f"{d}_Ddg"] = nc.declare_dram_parameter(f"{d}_Ddg", [NBLK, 128, 128], F16, isOutput=False)
    y_param = nc.declare_dram_parameter("y", [D_MODEL, L], F32, isOutput=True)

    with contextlib.ExitStack() as ctx:
        tc = ctx.enter_context(tile.TileContext(nc))
        pools = {
            "weights": ctx.enter_context(tc.tile_pool(name="weights", bufs=1)),
            "half": ctx.enter_context(tc.tile_pool(name="half", bufs=1)),
            "core": ctx.enter_context(tc.tile_pool(name="core", bufs=2)),
            "psum": ctx.enter_context(tc.tile_pool(name="psum", bufs=1, space="PSUM")),
            "psumy": ctx.enter_context(tc.tile_pool(name="psumy", bufs=1, space="PSUM")),
            "small": ctx.enter_context(tc.tile_pool(name="small", bufs=1)),
            "dram": ctx.enter_context(tc.tile_pool(name="dram", bufs=1, space="DRAM")),
        }
        identt = pools["weights"].tile([128, 128], F16, tag="ident", name="identt")
        nc.sync.dma_start(out=identt, in_=prm["ident"][:])
        ob = [pools["dram"].tile([128, L], F16, tag=f"ob{m}", name=f"ob{m}") for m in range(NMT)]
        for dirn in ("b", "f"):
            W = _load_weights(nc, pools["weights"], pools["small"], prm, dirn, identt)
            for lh in range(2):
                _emit_dir(nc, tc, pools, prm, W, dirn, lh, L, ob, y_param[:])
    nc.compile()
    return nc


@functools.lru_cache(maxsize=None)
def _get_nc(L):
    return build_nc(L)


def _prep_dir_weights(pfx, inputs):
    in_w = np.asarray(inputs[pfx + "_in_w"], np.float32)      # [1024, 256]
    conv_w = np.asarray(inputs[pfx + "_conv_w"], np.float32)  # [512, 4]
    conv_b = np.asarray(inputs[pfx + "_conv_b"], np.float32)  # [512]
    xproj_w = np.asarray(inputs[pfx + "_xproj_w"], np.float32)  # [48, 512]
    dt_w = np.asarray(inputs[pfx + "_dt_w"], np.float32)      # [512, 16]
    dt_b = np.asarray(inputs[pfx + "_dt_b"], np.float32)      # [512]
    A_log = np.asarray(inputs[pfx + "_A_log"], np.float32)    # [512, 16]
    Dp = np.asarray(inputs[pfx + "_D"], np.float32)           # [512]
    out_w = np.asarray(inputs[pfx + "_out_w"], np.float32)    # [256, 512]

    in_w_x = in_w[:D_INNER]        # [512, 256]
    in_w_z = in_w[D_INNER:]        # [512, 256]
    wx = np.zeros([2 * KCONV, 128, D_INNER], np.float16)
    for k in range(KCONV):
        for dmb in range(2):
            wx[k * 2 + dmb] = (
                in_w_x[:, dmb * 128:(dmb + 1) * 128] * conv_w[:, k:k + 1]
            ).T.astype(np.float16)
    wz = np.stack([
        in_w_z[:, dmb * 128:(dmb + 1) * 128].T for dmb in range(2)
    ]).astype(np.float16)          # [2, 128, 512]
    # permute xproj rows: dt rows 0..15 stay; then per group g the B/C rows
    # interleave as [B_g0, C_g0, B_g1, C_g1, ...] so each (g, j) pair is
    # adjacent for the broadcast DMA.
    perm = []
    for g in range(NG):
        for j in range(NJ):
            perm.append(DT_RANK + NJ * g + j)            # B row
            perm.append(DT_RANK + NSTATE + NJ * g + j)   # C row
    perm += list(range(DT_RANK))
    xproj_p = xproj_w[perm]
    wxp = np.stack([
        xproj_p[:, b * 128:(b + 1) * 128].T for b in range(NBLK)
    ]).astype(np.float16)          # [4, 128, 48]
    wdt = dt_w.T.astype(np.float16)  # [16, 512]
    wo = np.stack([
        0.5 * out_w[:, b * 128:(b + 1) * 128].T for b in range(NBLK)
    ]).astype(np.float16)          # [4, 128, 256]
    A = (-np.exp(A_log)).astype(np.float32)
    return {
        f"{pfx}_wx": wx,
        f"{pfx}_wz": wz,
        f"{pfx}_wxp": wxp,
        f"{pfx}_wdt": wdt,
        f"{pfx}_wo": wo,
        f"{pfx}_cb": conv_b.reshape(NBLK, 128, 1).astype(np.float32),
        f"{pfx}_dtb": dt_b.reshape(NBLK, 128, 1).astype(np.float32),
        f"{pfx}_A": A.reshape(NBLK, 128, NSTATE),
        f"{pfx}_D": Dp.reshape(NBLK, 128, 1).astype(np.float32),
        f"{pfx}_Ddg": np.stack([np.diag(Dp[b * 128:(b + 1) * 128])
                                for b in range(NBLK)]).astype(np.float16),
    }


def make_in_maps(inputs, L):
    hs = np.asarray(inputs["hidden_states"], np.float32)  # [B, L, 256]
    B = hs.shape[0]
    wmap = {"ident": np.eye(128, dtype=np.float16)}
    for pfx in ("f", "b"):
        wmap.update(_prep_dir_weights(pfx, inputs))
    in_maps = []
    for c in range(B):
        u = np.ascontiguousarray(hs[c].T)  # [256, L]
        pad = np.zeros([D_MODEL, KCONV - 1], np.float32)
        u_f = np.concatenate([pad, u], axis=1).astype(np.float16)
        u_b = np.concatenate([pad, u[:, ::-1]], axis=1).astype(np.float16)
        m = dict(wmap)
        m["u_f"] = u_f
        m["u_b"] = u_b
        in_maps.append(m)
    return in_maps


def run(inputs, trace=False, **kwargs):
    from concourse.bass_utils import run_bass_kernel_spmd

    hs = np.asarray(inputs["hidden_states"], np.float32)
    B, L, _ = hs.shape
    nc = _get_nc(L)
    in_maps = make_in_maps(inputs, L)
    res = run_bass_kernel_spmd(nc, in_maps, list(range(N_CORES))[:B],
                               trace=trace, **kwargs)
    out = np.stack([
        np.asarray(res.results[c]["y"]).T for c in range(B)
    ]).astype(np.float32)
    return out, res


def kernel(**inputs):
    return run(inputs)[0]


# revision 34
# speedup vs baseline: 1.0192x; 1.0192x over previous
"""Bidirectional Mamba block on 8 Trainium2 NeuronCores.

Strategy (v2)
-------------
Data-parallel over batch: each core runs one batch element (both
directions); no collectives.

Per core / per direction, d_inner=512 as 4 blocks of 128 partitions,
n=16 states, L=4096 in 2 halves of HL=2048:

  phase 1 (PE+ACT): x_conv = silu(conv1d(u @ in_w_x.T) + conv_b) with the
          depthwise conv folded into the input projection (4 taps x 2
          d_model tiles of host-combined weights); sz = silu(u @ in_w_z.T)
          bounced to DRAM until gating.
  phase 2 (PE+ACT): dbl = x_conv @ xproj.T with host-permuted rows so each
          state group's B/C rows are adjacent; delta = softplus(dt-proj +
          dt_b) via Exp+Ln (one ACT table set). B/C rows land in a small
          SBUF tile (bcsb) and are broadcast to 128 partitions by
          SBUF->SBUF DMA per (group, j) pair.
  phase 3: per (block, n): a = exp(A[:,n] * delta) on ACT at FD=2048;
          b = (delta*x_conv) * B (DVE, partly GPSIMD); h =
          tensor_tensor_scan(a, b) at FD=2048 chained across halves via
          f32 state tiles; p = h * C on GPSIMD; the n-reduction
          y = sum_n p runs on the PE as identity matmuls accumulating in
          PSUM (start/stop over the 16 planes).
  gating: gt = ((x_conv * D) + y_psum) * sz via one scalar_tensor_tensor
          + one tensor_tensor; out-proj on the PE (0.5 folded into out_w).

The backward direction runs first on a host-reversed input copy; its
out-proj result is stored to DRAM (f32) and combined, reversed, with the
forward direction's out-proj.
"""

import contextlib
import functools
import sys

for _p in ("/opt/trn_rl_repo",):
    if _p not in sys.path:
        sys.path.insert(0, _p)

import numpy as np

import concourse.bass as bass
import concourse.bacc as bacc
import concourse.mybir as mybir
import concourse.tile as tile

F16 = mybir.dt.float16
F32 = mybir.dt.float32
AOP = mybir.AluOpType
ACT = mybir.ActivationFunctionType

D_MODEL = 256
D_INNER = 512
NSTATE = 16
DT_RANK = 16
KCONV = 4
NBLK = D_INNER // 128   # 4 d_inner blocks
NMT = D_MODEL // 128    # 2 d_model tiles
NJ = 4                  # n per group
NG = NSTATE // NJ       # 4 groups
N_CORES = 8

L_FULL = 4096
T = 512                 # PSUM-sized chunk
N_CORES = 8

# which (g, j) bt muls go to GPSIMD instead of DVE (pt2 is always GPSIMD)
BT_GPSIMD_J = ()

SILU_NATIVE = True


def _patch_act_tables():
    """Keep Exp and Ln resolving to natural_log_exp_and_others so softplus
    (Exp+Ln) and the a-gen Exps never force ACT table reloads."""
    import concourse.bacc as _bacc
    import concourse.hw_specs as _hw

    if getattr(_bacc, "_mamba_act_patch", False):
        return
    real = _hw.get_activation_tables

    def patched(arch):
        tabs = dict(real(arch))
        keep = ("natural_log_exp_and_others", "silu_and_others")
        for nm in list(tabs):
            if nm not in keep:
                tabs[nm] = set()
        return tabs

    _bacc.get_activation_tables = patched
    _bacc._mamba_act_patch = True


def _emit_dir(nc, tc, pools, prm, W, dirn, lh, L, ob, y_param):
    HL = L // 2
    NCH = HL // T
    h0 = lh * HL
    wp, hp, core, ps, psy, sm, dram = (
        pools["weights"], pools["half"], pools["core"], pools["psum"],
        pools["psumy"], pools["small"], pools["dram"],
    )

    def dma(out, in_):
        nc.sync.dma_start(out=out, in_=in_)

    wx, wz, wxp, wdt, wo, cb, dtb, At, Dd, ident, state = (
        W['wx'], W['wz'], W['wxp'], W['wdt'], W['wo'], W['cb'], W['dtb'],
        W['A'], W['D'], W['ident'], W['state'])
    Ddg = W['Ddg']

    u_param = prm[f"u_{dirn}"]

    # ---- per-(direction, half) tensors; bufs=2 pipelines the two halves ----
    xc = [hp.tile([128, HL], F16, tag=f"xc{b}", name=f"xc{b}", bufs=2) for b in range(NBLK)]
    dl = [hp.tile([128, HL], F16, tag=f"dl{b}", name=f"dl{b}", bufs=2) for b in range(NBLK)]
    wh = [hp.tile([128, HL], F16, tag=f"wh{b}", name=f"wh{b}", bufs=2) for b in range(NBLK)]
    szd = [dram.tile([128, HL], F16, tag=f"szd{b}", name=f"szd{b}", bufs=2) for b in range(NBLK)]
    bcsb = hp.tile([2 * NSTATE, HL], F16, tag="bcsb", name="bcsb", bufs=2)
    bc = dram.tile([2 * NSTATE, HL], F16, tag="bc", name="bc", bufs=2)

    # ---- phase 1: x_conv (conv folded into in-proj); z lags one chunk ----
    def _emit_z(item):
        uu, s0 = item
        for b in range(NBLK):
            pz = ps.tile([128, T], F32, tag="pz", name="pz", bufs=1)
            for dmb in range(2):
                nc.tensor.matmul(
                    pz[:], wz[dmb][:, b * 128:(b + 1) * 128],
                    uu[dmb][:, KCONV - 1:KCONV - 1 + T],
                    start=(dmb == 0), stop=(dmb == 1),
                )
            szc = core.tile([128, T], F16, tag="szc", name="szc", bufs=1)
            nc.scalar.activation(out=szc[:], in_=pz[:], func=ACT.Silu)
            dma(szd[b][:, s0:s0 + T], szc[:])

    for ci in range(NCH):
        t0 = h0 + ci * T
        s0 = ci * T
        u0 = core.tile([128, T + KCONV - 1], F16, tag="u0", name="u0")
        u1 = core.tile([128, T + KCONV - 1], F16, tag="u1", name="u1")
        dma(u0, u_param[0:128, t0:t0 + T + KCONV - 1])
        dma(u1, u_param[128:256, t0:t0 + T + KCONV - 1])
        uu = (u0, u1)
        for b in range(NBLK):
            px = ps.tile([128, T], F32, tag="px", name="px", bufs=1)
            for kb in range(2 * KCONV):
                k, dmb = divmod(kb, 2)
                nc.tensor.matmul(
                    px[:], wx[kb][:, b * 128:(b + 1) * 128],
                    uu[dmb][:, k:k + T],
                    start=(kb == 0), stop=(kb == 2 * KCONV - 1),
                )
            nc.scalar.activation(
                out=xc[b][:, s0:s0 + T], in_=px[:], func=ACT.Silu,
                bias=cb[b][:], scale=1.0,
            )
        _emit_z((uu, s0))

    # ---- phase 2: dbl (dt/B/C rows, B/C host-permuted) and delta ----
    for ci in range(NCH):
        s0 = ci * T
        pd = ps.tile([128, T], F32, tag="pd", name="pd")
        for b in range(NBLK):
            nc.tensor.matmul(
                pd[0:DT_RANK + 2 * NSTATE, :], wxp[b][:], xc[b][:, s0:s0 + T],
                start=(b == 0), stop=(b == NBLK - 1),
            )
        dt16 = core.tile([DT_RANK, T], F16, tag="dt16", name="dt16", bufs=1)
        nc.scalar.activation(out=dt16[:],
                             in_=pd[2 * NSTATE:2 * NSTATE + DT_RANK, :],
                             func=ACT.Copy)
        nc.scalar.activation(out=bcsb[:, s0:s0 + T],
                             in_=pd[0:2 * NSTATE, :],
                             func=ACT.Copy)
        for b in range(NBLK):
            pt = ps.tile([128, T], F32, tag="pd", name="pt")
            nc.tensor.matmul(
                pt[:], wdt[:, b * 128:(b + 1) * 128], dt16[:],
                start=True, stop=True,
            )
            et = core.tile([128, T], F32, tag="et", name="et", bufs=1)
            nc.scalar.activation(out=et[:], in_=pt[:], func=ACT.Exp,
                                 bias=dtb[b][:], scale=1.0)
            nc.scalar.activation(out=dl[b][:, s0:s0 + T], in_=et[:],
                                 func=ACT.Ln, bias=1.0, scale=1.0)
        dma(bc[:, s0:s0 + T], bcsb[:, s0:s0 + T])

    # ---- phase 3: selective scan; n-reduce on PE into PSUM ----
    gts = []
    P_SIZES = (HL // 2, HL // 2) if (dirn == "b" and lh == 0) else (HL,)
    NP = len(P_SIZES)
    for b in range(NBLK):
        yacc = [psy.tile([128, T], F32, tag=f"yac{c}", name=f"yac{c}")
                for c in range(NCH)]
        state2 = wp.tile([128, NSTATE], F32, tag=f"st2_{b}", name=f"st2_{b}")
        for p in range(NP):
            PL = P_SIZES[p]
            p0 = sum(P_SIZES[:p])
            for g in range(NG):
                for j in range(NJ):
                    n = g * NJ + j
                    bbt = core.tile([128, HL], F16, tag="bbt", name="bbt",
                                    bufs=2)
                    cbt = core.tile([128, HL], F16, tag="cbt", name="cbt",
                                    bufs=2)
                    at = core.tile([128, HL], F16, tag="at", name="at",
                                   bufs=3)
                    bt = core.tile([128, HL], F16, tag="bt", name="bt",
                                   bufs=2)
                    ht = core.tile([128, HL], F16, tag="ht", name="ht",
                                   bufs=3)
                    pt2 = core.tile([128, HL], F16, tag="pt2", name="pt2",
                                    bufs=1)
                    bc_ap = bc[:]
                    if g == 0 and j == 0:
                        nc.vector.tensor_tensor(
                            wh[b][:, p0:p0 + PL], dl[b][:, p0:p0 + PL],
                            xc[b][:, p0:p0 + PL], AOP.mult)
                    dma(bbt[:, p0:p0 + PL], bass.AP(
                        tensor=bc_ap.tensor,
                        offset=bc_ap.offset + (2 * n) * HL + p0,
                        ap=[[0, 128], [1, PL]],
                    ))
                    dma(cbt[:, p0:p0 + PL], bass.AP(
                        tensor=bc_ap.tensor,
                        offset=bc_ap.offset + (2 * n + 1) * HL + p0,
                        ap=[[0, 128], [1, PL]],
                    ))
                    nc.scalar.activation(
                        out=at[:, p0:p0 + PL], in_=dl[b][:, p0:p0 + PL],
                        func=ACT.Exp, scale=At[b][:, n:n + 1],
                    )
                    nc.vector.tensor_tensor(
                        bt[:, p0:p0 + PL], wh[b][:, p0:p0 + PL],
                        bbt[:, p0:p0 + PL], AOP.mult)
                    if p == 0:
                        init = 0.0 if lh == 0 else state[b][:, n:n + 1]
                    else:
                        init = state2[:, n:n + 1]
                    nc.vector.tensor_tensor_scan(
                        ht[:, p0:p0 + PL], at[:, p0:p0 + PL],
                        bt[:, p0:p0 + PL], init, AOP.mult, AOP.add)
                    if p < NP - 1:
                        nc.gpsimd.tensor_copy(
                            state2[:, n:n + 1],
                            ht[:, p0 + PL - 1:p0 + PL])
                    if lh == 0 and p == NP - 1:
                        nc.gpsimd.tensor_copy(
                            state[b][:, n:n + 1], ht[:, HL - 1:HL])
                    nc.vector.tensor_tensor(
                        pt2[:, p0:p0 + PL], ht[:, p0:p0 + PL],
                        cbt[:, p0:p0 + PL], AOP.mult)
                    for c in range(p0 // T, (p0 + PL) // T):
                        nc.tensor.matmul(
                            yacc[c][:], ident[:], pt2[:, c * T:(c + 1) * T],
                            start=(g == 0 and j == 0), stop=False,
                        )
        # gating for this block: yacc += diag(D) @ xc on PE, then * sz
        gt = core.tile([128, HL], F16, tag=f"gt{b}", name=f"gt{b}", bufs=1)
        szt = core.tile([128, HL], F16, tag="szt", name="szt", bufs=2)
        dma(szt, szd[b][:])
        for c in range(NCH):
            nc.tensor.matmul(
                yacc[c][:], Ddg[b][:], xc[b][:, c * T:(c + 1) * T],
                start=False, stop=True,
            )
            nc.vector.tensor_tensor(
                gt[:, c * T:(c + 1) * T], yacc[c][:],
                szt[:, c * T:(c + 1) * T], AOP.mult)
        gts.append(gt)
    # ---- out-proj ----
    last_unit = (dirn == "f" and lh == 1)
    for c in range(NCH):
        t0 = h0 + c * T
        for mt in range(NMT):
            po_tag = "px" if (last_unit and (c * NMT + mt) % 2 == 1) else "po"
            po = ps.tile([128, T], F32, tag=po_tag, name="po", bufs=1)
            for b in range(NBLK):
                nc.tensor.matmul(
                    po[:], wo[b][:, mt * 128:(mt + 1) * 128],
                    gts[b][:, c * T:(c + 1) * T],
                    start=(b == 0),
                    stop=(b == NBLK - 1 and dirn == "b"),
                )
            if dirn == "b":
                obs = core.tile([128, T], F16, tag="obs", name="obs", bufs=1)
                nc.scalar.activation(out=obs[:], in_=po[:], func=ACT.Copy)
                dma(ob[mt][:, t0:t0 + T], obs[:])
            else:
                cb_rev = (L // T) - 1 - (t0 // T)
                obs = core.tile([128, T], F16, tag="obs", name="obs", bufs=1)
                dma(obs, ob[mt][:, cb_rev * T:(cb_rev + 1) * T])
                nc.tensor.matmul(
                    po[:], ident[:], obs[:, ::-1],
                    start=False, stop=True,
                )
                oo = core.tile([128, T], F32, tag="oo", name="oo", bufs=1)
                nc.scalar.activation(out=oo[:], in_=po[:], func=ACT.Copy)
                dma(y_param[mt * 128:(mt + 1) * 128, t0:t0 + T], oo[:])


def _load_weights(nc, wp, sm, prm, dirn, ident):
    def dma(out, in_):
        nc.sync.dma_start(out=out, in_=in_)
    W = {"ident": ident}
    W["wx"] = []
    for i in range(2 * KCONV):
        t = wp.tile([128, D_INNER], F16, tag=f"wx{i}", name=f"wx{i}")
        dma(t, prm[f"{dirn}_wx"][i])
        W["wx"].append(t)
    W["wz"] = []
    for i in range(2):
        t = wp.tile([128, D_INNER], F16, tag=f"wz{i}", name=f"wz{i}")
        dma(t, prm[f"{dirn}_wz"][i])
        W["wz"].append(t)
    W["wxp"] = []
    for b in range(NBLK):
        t = wp.tile([128, DT_RANK + 2 * NSTATE], F16, tag=f"wxp{b}", name=f"wxp{b}")
        dma(t, prm[f"{dirn}_wxp"][b])
        W["wxp"].append(t)
    W["wdt"] = wp.tile([DT_RANK, D_INNER], F16, tag="wdt", name="wdt")
    dma(W["wdt"], prm[f"{dirn}_wdt"][:])
    W["wo"] = []
    for b in range(NBLK):
        t = wp.tile([128, D_MODEL], F16, tag=f"wo{b}", name=f"wo{b}")
        dma(t, prm[f"{dirn}_wo"][b])
        W["wo"].append(t)
    W["Ddg"] = []
    for b in range(NBLK):
        t = wp.tile([128, 128], F16, tag=f"Ddg{b}", name=f"Ddg{b}")
        dma(t, prm[f"{dirn}_Ddg"][b])
        W["Ddg"].append(t)
    for key, pname, width in (("cb", "cb", 1), ("dtb", "dtb", 1),
                              ("A", "A", NSTATE), ("D", "D", 1)):
        W[key] = []
        for b in range(NBLK):
            t = wp.tile([128, width], F32, tag=f"{key}{b}", name=f"{key}{b}")
            dma(t, prm[f"{dirn}_{pname}"][b])
            W[key].append(t)
    W["state"] = [sm.tile([128, NSTATE], F32, tag=f"st{b}", name=f"st{b}")
                  for b in range(NBLK)]
    return W


def build_nc(L):
    _patch_act_tables()
    nc = bacc.Bacc("TRN2", target_bir_lowering=False, debug=False)
    prm = {}
    prm["u_f"] = nc.declare_dram_parameter("u_f", [D_MODEL, L + KCONV - 1], F16, isOutput=False)
    prm["u_b"] = nc.declare_dram_parameter("u_b", [D_MODEL, L + KCONV - 1], F16, isOutput=False)
    prm["ident"] = nc.declare_dram_parameter("ident", [128, 128], F16, isOutput=False)
    for d in ("f", "b"):
        prm[f"{d}_wx"] = nc.declare_dram_parameter(f"{d}_wx", [2 * KCONV, 128, D_INNER], F16, isOutput=False)
        prm[f"{d}_wz"] = nc.declare_dram_parameter(f"{d}_wz", [2, 128, D_INNER], F16, isOutput=False)
        prm[f"{d}_wxp"] = nc.declare_dram_parameter(f"{d}_wxp", [NBLK, 128, DT_RANK + 2 * NSTATE], F16, isOutput=False)
        prm[f"{d}_wdt"] = nc.declare_dram_parameter(f"{d}_wdt", [DT_RANK, D_INNER], F16, isOutput=False)
        prm[f"{d}_wo"] = nc.declare_dram_parameter(f"{d}_wo", [NBLK, 128, D_MODEL], F16, isOutput=False)
        prm[f"{d}_cb"] = nc.declare_dram_parameter(f"{d}_cb", [NBLK, 128, 1], F32, isOutput=False)
        prm[f"{d}_dtb"] = nc.declare_dram_parameter(f"{d}_dtb", [NBLK, 128, 1], F32, isOutput=False)
        prm[f"{d}_A"] = nc.declare_dram_parameter(f"{d}_A", [NBLK, 128, NSTATE], F32, isOutput=False)
        prm[f"{d}_D"] = nc.declare_dram_parameter(f"{d}_D", [NBLK, 128, 1], F32, isOutput=False)
        prm[f"{d}_Ddg"] = nc.declare_dram_parameter(f"{d}_Ddg", [NBLK, 128, 128], F16, isOutput=False)
    y_param = nc.declare_dram_parameter("y", [D_MODEL, L], F32, isOutput=True)

    with contextlib.ExitStack() as ctx:
        tc = ctx.enter_context(tile.TileContext(nc))
        pools = {
            "weights": ctx.enter_context(tc.tile_pool(name="weights", bufs=1)),
            "half": ctx.enter_context(tc.tile_pool(name="half", bufs=1)),
            "core": ctx.enter_context(tc.tile_pool(name="core", bufs=2)),
            "psum": ctx.enter_context(tc.tile_pool(name="psum", bufs=1, space="PSUM")),
            "psumy": ctx.enter_context(tc.tile_pool(name="psumy", bufs=1, space="PSUM")),
            "small": ctx.enter_context(tc.tile_pool(name="small", bufs=1)),
            "dram": ctx.enter_context(tc.tile_pool(name="dram", bufs=1, space="DRAM")),
        }
        identt = pools["weights"].tile([128, 128], F16, tag="ident", name="identt")
        nc.sync.dma_start(out=identt, in_=prm["ident"][:])
        ob = [pools["dram"].tile([128, L], F16, tag=f"ob{m}", name=f"ob{m}") for m in range(NMT)]
        for dirn in ("b", "f"):
            W = _load_weights(nc, pools["weights"], pools["small"], prm, dirn, identt)
            for lh in range(2):
                _emit_dir(nc, tc, pools, prm, W, dirn, lh, L, ob, y_param[:])
    nc.compile()
    return nc


@functools.lru_cache(maxsize=None)
def _get_nc(L):
    return build_nc(L)


def _prep_dir_weights(pfx, inputs):
    in_w = np.asarray(inputs[pfx + "_in_w"], np.float32)      # [1024, 256]
    conv_w = np.asarray(inputs[pfx + "_conv_w"], np.float32)  # [512, 4]
    conv_b = np.asarray(inputs[pfx + "_conv_b"], np.float32)  # [512]
    xproj_w = np.asarray(inputs[pfx + "_xproj_w"], np.float32)  # [48, 512]
    dt_w = np.asarray(inputs[pfx + "_dt_w"], np.float32)      # [512, 16]
    dt_b = np.asarray(inputs[pfx + "_dt_b"], np.float32)      # [512]
    A_log = np.asarray(inputs[pfx + "_A_log"], np.float32)    # [512, 16]
    Dp = np.asarray(inputs[pfx + "_D"], np.float32)           # [512]
    out_w = np.asarray(inputs[pfx + "_out_w"], np.float32)    # [256, 512]

    in_w_x = in_w[:D_INNER]        # [512, 256]
    in_w_z = in_w[D_INNER:]        # [512, 256]
    wx = np.zeros([2 * KCONV, 128, D_INNER], np.float16)
    for k in range(KCONV):
        for dmb in range(2):
            wx[k * 2 + dmb] = (
                in_w_x[:, dmb * 128:(dmb + 1) * 128] * conv_w[:, k:k + 1]
            ).T.astype(np.float16)
    wz = np.stack([
        in_w_z[:, dmb * 128:(dmb + 1) * 128].T for dmb in range(2)
    ]).astype(np.float16)          # [2, 128, 512]
    # permute xproj rows: dt rows 0..15 stay; then per group g the B/C rows
    # interleave as [B_g0, C_g0, B_g1, C_g1, ...] so each (g, j) pair is
    # adjacent for the broadcast DMA.
    perm = []
    for g in range(NG):
        for j in range(NJ):
            perm.append(DT_RANK + NJ * g + j)            # B row
            perm.append(DT_RANK + NSTATE + NJ * g + j)   # C row
    perm += list(range(DT_RANK))
    xproj_p = xproj_w[perm]
    wxp = np.stack([
        xproj_p[:, b * 128:(b + 1) * 128].T for b in range(NBLK)
    ]).astype(np.float16)          # [4, 128, 48]
    wdt = dt_w.T.astype(np.float16)  # [16, 512]
    wo = np.stack([
        0.5 * out_w[:, b * 128:(b + 1) * 128].T for b in range(NBLK)
    ]).astype(np.float16)          # [4, 128, 256]
    A = (-np.exp(A_log)).astype(np.float32)
    return {
        f"{pfx}_wx": wx,
        f"{pfx}_wz": wz,
        f"{pfx}_wxp": wxp,
        f"{pfx}_wdt": wdt,
        f"{pfx}_wo": wo,
        f"{pfx}_cb": conv_b.reshape(NBLK, 128, 1).astype(np.float32),
        f"{pfx}_dtb": dt_b.reshape(NBLK, 128, 1).astype(np.float32),
        f"{pfx}_A": A.reshape(NBLK, 128, NSTATE),
        f"{pfx}_D": Dp.reshape(NBLK, 128, 1).astype(np.float32),
        f"{pfx}_Ddg": np.stack([np.diag(Dp[b * 128:(b + 1) * 128])
                                for b in range(NBLK)]).astype(np.float16),
    }


def make_in_maps(inputs, L):
    hs = np.asarray(inputs["hidden_states"], np.float32)  # [B, L, 256]
    B = hs.shape[0]
    wmap = {"ident": np.eye(128, dtype=np.float16)}
    for pfx in ("f", "b"):
        wmap.update(_prep_dir_weights(pfx, inputs))
    in_maps = []
    for c in range(B):
        u = np.ascontiguousarray(hs[c].T)  # [256, L]
        pad = np.zeros([D_MODEL, KCONV - 1], np.float32)
        u_f = np.concatenate([pad, u], axis=1).astype(np.float16)
        u_b = np.concatenate([pad, u[:, ::-1]], axis=1).astype(np.float16)
        m = dict(wmap)
        m["u_f"] = u_f
        m["u_b"] = u_b
        in_maps.append(m)
    return in_maps


def run(inputs, trace=False, **kwargs):
    from concourse.bass_utils import run_bass_kernel_spmd

    hs = np.asarray(inputs["hidden_states"], np.float32)
    B, L, _ = hs.shape
    nc = _get_nc(L)
    in_maps = make_in_maps(inputs, L)
    res = run_bass_kernel_spmd(nc, in_maps, list(range(N_CORES))[:B],
                               trace=trace, **kwargs)
    out = np.stack([
        np.asarray(res.results[c]["y"]).T for c in range(B)
    ]).astype(np.float32)
    return out, res


def kernel(**inputs):
    return run(inputs)[0]
